# revision 2
# baseline (speedup 1.0000x reference)
"""JK-GAMLP forward on 8 Trainium2 NeuronCores (Bass/Tile) — fp8 DR, 8-stage.

Per core: node tiles of 512 (4 blocks of 128 on partitions).  The JK-MLP +
attention-score path runs in fp8e4m3 DoubleRow matmuls (K=256/instr, 0.5
cyc/row); weights pre-scaled by 32 (fp8 subnormal headroom), divided back
out in the activation scales.  The output path (hop aggregation -> output
FFN) is f32-accumulate with bf16 operands; output is stored transposed
([NCLS, N]) so store descriptors are 2KB, un-transposed on the host.

An 8-stage software pipeline: every engine's in-order queue gets work
whose cross-engine inputs completed in a PREVIOUS iteration, so nothing
stalls at queue heads.  Stage schedule per loop iteration t:
  LOAD(t) -> TRANS(t-1) -> AGG(t-5, Pool+DVE) -> H1(t-2) -> SCORE(t-3) ->
  AGGT(t-6) -> OUT(t-7) -> SOFT(t-4) -> JK(t-2)
"""
import numpy as np

import concourse.bacc as bacc
import concourse.mybir as mybir
import concourse.tile as tile
from concourse.bass_utils import run_bass_kernel_spmd

AF = mybir.ActivationFunctionType
ALU = mybir.AluOpType
AX = mybir.AxisListType
F32 = mybir.dt.float32
F32R = mybir.dt.float32r
BF16 = mybir.dt.bfloat16
FP8 = mybir.dt.float8e4
DR = mybir.MatmulPerfMode.DoubleRowSwInterleave

HOPS, F, HID, NCLS = 8, 128, 256, 64
N = 100000
NCORES = 8
NPC = 12544                       # nodes per core (padded: 8*12544 = 100352)
TILES = [(i * 512, 512) for i in range(24)] + [(12288, 256)]
WS = 32.0                         # fp8 weight pre-scale
ACT_XT_HOPS = (0, 2, 4, 6)           # xt psum->sbuf copies on ACT; rest on DVE

_CACHE = {}


def _build_program(loop_k=None, unroll=1):
    nc = bacc.Bacc("TRN2", target_bir_lowering=False, debug=False,
                   num_devices=NCORES)

    feats = nc.dram_tensor("feats", [HOPS, NPC, F], F32R, kind="ExternalInput")
    W_jk1 = nc.dram_tensor("W_jk1", [HOPS * F, HID], F32, kind="ExternalInput")
    W_jk2 = nc.dram_tensor("W_jk2", [HID, HID], F32, kind="ExternalInput")
    w_att_ref = nc.dram_tensor("w_att_ref", [HID], F32, kind="ExternalInput")
    w_att_x = nc.dram_tensor("w_att_x", [F], F32, kind="ExternalInput")
    W_o1 = nc.dram_tensor("W_o1", [F, HID], F32, kind="ExternalInput")
    W_o2 = nc.dram_tensor("W_o2", [HID, NCLS], F32, kind="ExternalInput")
    a_jk = nc.dram_tensor("a_jk", [1, 1], F32, kind="ExternalInput")
    a_main = nc.dram_tensor("a_main", [1, 1], F32, kind="ExternalInput")
    a_out = nc.dram_tensor("a_out", [1, 1], F32, kind="ExternalInput")
    outT = nc.dram_tensor("outT", [NCLS, NPC], F32, kind="ExternalOutput")

    ident = nc.inline_tensor(np.eye(128, dtype=np.float32), name="ident")

    with tile.TileContext(nc) as tc:
        with tc.tile_pool(name="const", bufs=1) as cpool, \
             tc.tile_pool(name="x", bufs=34) as xpool, \
             tc.tile_pool(name="xt", bufs=13) as xtpool, \
             tc.tile_pool(name="act", bufs=3) as actpool, \
             tc.tile_pool(name="sm", bufs=4) as smpool, \
             tc.tile_pool(name="aggtmp", bufs=2) as aggtmpool, \
             tc.tile_pool(name="aggfin", bufs=12) as aggfpool, \
             tc.tile_pool(name="xt_ps", bufs=2, space="PSUM") as xtps, \
             tc.tile_pool(name="mm_ps", bufs=2, space="PSUM") as mmps, \
             tc.tile_pool(name="mm3_ps", bufs=1, space="PSUM") as mm3ps:

            # ---------------- setup: weights + constants ----------------
            from concourse import library_config
            nc.gpsimd.load_library(library_config.standard)
            id_sb = cpool.tile([128, 128], F32)
            nc.sync.dma_start(id_sb[:], ident[:])
            id_bf = cpool.tile([128, 128], BF16)
            nc.vector.tensor_copy(id_bf[:], id_sb[:])

            w1f = cpool.tile([128, HOPS, 2, 128], F32)
            nc.sync.dma_start(
                w1f[:], W_jk1.ap().rearrange("(h p) (m c) -> p h m c",
                                             p=128, m=2))
            # SwInterleave DR layout [p, j, m, 2c]: stored col 2v+i =
            # k-tile i (hop 2j+i), logical column 127-v, per the hardware
            # dual-fp8 LdWeights ordering.
            w1q = cpool.tile([128, 4, 2, 256], FP8)
            for j in range(4):
                for m in range(2):
                    for i in range(2):
                        nc.vector.tensor_scalar(
                            w1q[:, j, m, i::2],
                            w1f[:, 2 * j + i, m, ::-1],
                            WS, None, op0=ALU.mult)

            w2f = cpool.tile([128, 2, 2, 128], F32)
            nc.sync.dma_start(
                w2f[:], W_jk2.ap().rearrange("(k p) (m c) -> p k m c",
                                             p=128, m=2))
            # SwInterleave DR layout [p, m, 2c]
            w2q = cpool.tile([128, 2, 256], FP8)
            for m in range(2):
                for i in range(2):
                    nc.vector.tensor_scalar(
                        w2q[:, m, i::2], w2f[:, i, m, ::-1],
                        WS, None, op0=ALU.mult)

            wo1f = cpool.tile([128, 2, 128], F32)
            nc.sync.dma_start(
                wo1f[:], W_o1.ap().rearrange("p (m c) -> p m c", m=2))
            wo1b = cpool.tile([128, 2, 128], BF16)
            nc.vector.tensor_copy(wo1b[:], wo1f[:])

            wo2f = cpool.tile([128, 2, NCLS], F32)
            nc.sync.dma_start(
                wo2f[:], W_o2.ap().rearrange("(k p) c -> p k c", p=128))
            wo2b = cpool.tile([128, 2, NCLS], BF16)
            nc.vector.tensor_copy(wo2b[:], wo2f[:])

            wreff = cpool.tile([128, 2], F32)
            nc.sync.dma_start(wreff[:],
                              w_att_ref.ap().rearrange("(k p) -> p k", p=128))
            # SwInterleave, M padded to 128 (matches the known-good w1q
            # LdWeights shape): logical cols 0..7 = wref chunk i
            # (replicated), rest zero; stored col 2v+i = logical 127-v
            wrefq = cpool.tile([128, 256], FP8)
            nc.vector.memset(wrefq[:], 0.0)
            for i in range(2):
                for v in range(120, 128):
                    nc.vector.tensor_scalar(wrefq[:, 2 * v + i:2 * v + i + 1],
                                            wreff[:, i:i + 1], WS, None,
                                            op0=ALU.mult)

            watxf = cpool.tile([128, 1], F32)
            nc.sync.dma_start(watxf[:],
                              w_att_x.ap().rearrange("(p o) -> p o", o=1))
            # SwInterleave [p, j, 256], M=128: hop 2j+i diag at logical
            # col 2j+i -> stored col 2*(127-(2j+i))+i = 254-4j-i
            watxq = cpool.tile([128, 4, 256], FP8)
            nc.vector.memset(watxq[:], 0.0)
            for j in range(4):
                for i in range(2):
                    col = 254 - 4 * j - i
                    nc.vector.tensor_scalar(watxq[:, j, col:col + 1],
                                            watxf[:], WS, None, op0=ALU.mult)

            al_f = cpool.tile([1, 3], F32)
            nc.sync.dma_start(al_f[0:1, 0:1], a_jk[:])
            nc.sync.dma_start(al_f[0:1, 1:2], a_main[:])
            nc.sync.dma_start(al_f[0:1, 2:3], a_out[:])
            ones_sb = cpool.tile([1, 128], F32)
            nc.vector.memset(ones_sb[:], 1.0)
            half_col = cpool.tile([128, 1], F32)
            nc.vector.memset(half_col[:], 0.5)
            al_ps = mmps.tile([128, 3], F32, tag="mm")
            nc.tensor.matmul(al_ps[:], ones_sb[:], al_f[:],
                             start=True, stop=True)
            alpha = cpool.tile([128, 3], F32)
            nc.scalar.activation(alpha[:], al_ps[:], AF.Copy)

            # ---------------- pipeline stages ----------------

            def s_load(st):
                n0, TT = st["n0"], st["TT"]
                B = TT // 128
                x_blocks = []
                for b in range(B):
                    xb = xpool.tile([128, HOPS, 128], F32R, tag="x",
                                    name=f"xb{b}")
                    nc.sync.dma_start(
                        xb[:],
                        feats.ap()[:, n0 + b * 128:n0 + (b + 1) * 128, :]
                        .rearrange("h p f -> p h f"))
                    x_blocks.append(xb)
                st["x"] = x_blocks

            def s_trans(st):
                TT = st["TT"]
                B = TT // 128
                x = st["x"]
                xt_pairs = [xtpool.tile([128, 2, 512], FP8, tag="xt",
                                        name=f"xtp{j}")
                            for j in range(4)]
                for h in range(HOPS):
                    ps = xtps.tile([128, 512], F32R, tag="xtps")
                    for b in range(B):
                        nc.tensor.transpose(ps[:, b * 128:(b + 1) * 128],
                                            x[b][:, h, :],
                                            id_sb[:].bitcast(F32R))
                    dst = xt_pairs[h // 2][:, h % 2, 0:TT]
                    if h in ACT_XT_HOPS:
                        nc.scalar.activation(dst, ps[:, 0:TT], AF.Copy)
                    else:
                        nc.vector.tensor_copy(dst, ps[:, 0:TT])
                st["xt"] = xt_pairs

            def s_agg(st):
                TT = st["TT"]
                B = TT // 128
                x, ew = st["x"], st["ew"]
                agg_last = [None] * B
                # Pool first: lead-in mults for the DVE blocks (so DVE's
                # chains find them ready), then Pool's own block-0 chain.
                # Chain scratch ping-pongs within per-block tags: all WARs
                # are same-engine (free); only the lead-ins cross engines.
                dve_cur = {}
                for b in range(1, B):
                    cur = aggtmpool.tile([128, 128], F32, tag="agg0", bufs=6,
                                         name=f"agg0_{b}")
                    nc.gpsimd.tensor_scalar(cur[:], x[b][:, 0, :].bitcast(F32),
                                            ew[:, b, 0:1], None, op0=ALU.mult)
                    dve_cur[b] = cur
                # Pool block 0: tensor_scalar mult + tensor_tensor add chain
                cur = aggtmpool.tile([128, 128], F32, tag="poolc", bufs=2)
                nc.gpsimd.tensor_scalar(cur[:], x[0][:, 0, :].bitcast(F32),
                                        ew[:, 0, 0:1], None, op0=ALU.mult)
                for h in range(1, HOPS):
                    t_h = aggtmpool.tile([128, 128], F32, tag="poolt", bufs=2)
                    nc.gpsimd.tensor_scalar(t_h[:], x[0][:, h, :].bitcast(F32),
                                            ew[:, 0, h:h + 1], None,
                                            op0=ALU.mult)
                    if h == HOPS - 1:
                        nxt = aggfpool.tile([128, 128], BF16, tag="aggf")
                    else:
                        nxt = aggtmpool.tile([128, 128], F32, tag="poolc",
                                             bufs=2)
                    nc.gpsimd.tensor_tensor(nxt[:], cur[:], t_h[:], op=ALU.add)
                    cur = nxt
                agg_last[0] = cur
                # DVE blocks: fused scalar_tensor_tensor chains from h1
                for b in range(1, B):
                    cur = dve_cur[b]
                    for h in range(1, HOPS):
                        if h == HOPS - 1:
                            nxt = aggfpool.tile([128, 128], BF16, tag="aggf")
                        else:
                            nxt = aggtmpool.tile([128, 128], F32,
                                                 tag=f"aggd{b}", bufs=2,
                                                 name=f"aggd{b}_{h}")
                        nc.vector.scalar_tensor_tensor(
                            nxt[:], x[b][:, h, :].bitcast(F32),
                            ew[:, b, h:h + 1], cur[:],
                            op0=ALU.mult, op1=ALU.add)
                        cur = nxt
                    agg_last[b] = cur
                st["agg"] = agg_last

            def s_h1(st):
                TT = st["TT"]
                xt_pairs = st["xt"]
                h1_pair = actpool.tile([128, 2, 512], FP8, tag="h1", bufs=2)
                ps = mmps.tile([128, 1024], F32, tag="mm")
                for m in range(2):
                    for j in range(4):
                        nc.tensor.matmul(ps[:, m * 512:m * 512 + TT],
                                         w1q[:, j, m, :],
                                         xt_pairs[j][:, :, 0:TT],
                                         start=(j == 0), stop=(j == 3),
                                         perf_mode=DR)
                if TT == 512:
                    nc.scalar.activation(h1_pair[:].rearrange("p m c -> p (m c)"),
                                         ps[:], AF.Prelu, alpha=alpha[:, 0:1],
                                         scale=1.0 / WS)
                else:
                    for m in range(2):
                        nc.scalar.activation(h1_pair[:, m, 0:TT],
                                             ps[:, m * 512:m * 512 + TT],
                                             AF.Prelu, alpha=alpha[:, 0:1],
                                             scale=1.0 / WS)
                st["h1"] = h1_pair

            def s_jk(st):
                TT = st["TT"]
                h1_pair = st["h1"]
                jk_pair = actpool.tile([128, 2, 512], FP8, tag="jk", bufs=2)
                ps = mmps.tile([128, 1024], F32, tag="mm")
                for m in range(2):
                    nc.tensor.matmul(ps[:, m * 512:m * 512 + TT], w2q[:, m, :],
                                     h1_pair[:, :, 0:TT],
                                     start=True, stop=True, perf_mode=DR)
                if TT == 512:
                    nc.scalar.activation(jk_pair[:].rearrange("p m c -> p (m c)"),
                                         ps[:], AF.Prelu, alpha=alpha[:, 1:2],
                                         scale=1.0 / WS)
                else:
                    for m in range(2):
                        nc.scalar.activation(jk_pair[:, m, 0:TT],
                                             ps[:, m * 512:m * 512 + TT],
                                             AF.Prelu, alpha=alpha[:, 1:2],
                                             scale=1.0 / WS)
                st["jk"] = jk_pair

            def s_score(st):
                TT = st["TT"]
                xt_pairs, jk_pair = st["xt"], st["jk"]
                s_tile = mmps.tile([128, 1024], F32, tag="mm")
                s_ps = s_tile[0:128, 0:TT]
                nc.tensor.matmul(s_ps, wrefq[:], jk_pair[:, :, 0:TT],
                                 start=True, stop=False, perf_mode=DR,
                                 skip_group_check=True)
                for j in range(4):
                    nc.tensor.matmul(s_ps, watxq[:, j, :],
                                     xt_pairs[j][:, :, 0:TT],
                                     start=False, stop=(j == 3), perf_mode=DR,
                                     skip_group_check=True)
                sg = smpool.tile([8, 512], F32, tag="sg")
                nc.scalar.activation(sg[0:8, 0:TT], s_tile[0:8, 0:TT],
                                     AF.Tanh, scale=0.5 / WS)
                st["sg"] = sg

            def s_soft(st):
                TT = st["TT"]
                B = TT // 128
                sg = st["sg"]
                e_ps = mmps.tile([128, 1024], F32, tag="mm")
                for b in range(B):
                    nc.tensor.transpose(e_ps[:, b * 8:(b + 1) * 8],
                                        sg[0:8, b * 128:(b + 1) * 128],
                                        id_sb[0:8, 0:8])
                e_sb = smpool.tile([128, 4, 8], F32, tag="e")
                nc.scalar.activation(e_sb[:, 0:B, :], e_ps[:, 0:B * 8],
                                     AF.Exp, scale=0.5, bias=half_col[:])
                esum = smpool.tile([128, 4], F32, tag="esum")
                nc.vector.tensor_reduce(esum[:, 0:B], e_sb[:, 0:B, :],
                                        axis=AX.X, op=ALU.add)
                r_sb = smpool.tile([128, 4], F32, tag="r")
                nc.vector.reciprocal(r_sb[:, 0:B], esum[:, 0:B])
                ew = smpool.tile([128, 4, 8], F32, tag="ew")
                for b in range(B):
                    nc.gpsimd.tensor_scalar(ew[:, b, :], e_sb[:, b, :],
                                            r_sb[:, b:b + 1], None,
                                            op0=ALU.mult)
                st["ew"] = ew

            def s_aggt(st):
                TT = st["TT"]
                B = TT // 128
                a_ps = mm3ps.tile([128, 1024], BF16, tag="mm3")
                for b in range(B):
                    nc.tensor.transpose(a_ps[:, b * 128:(b + 1) * 128],
                                        st["agg"][b][:], id_bf[:])
                aggt = actpool.tile([128, 512], BF16, tag="aggt", bufs=3)
                nc.vector.tensor_copy(aggt[:, 0:TT], a_ps[:, 0:TT])
                st["aggt"] = aggt

            def s_out(st):
                n0, TT = st["n0"], st["TT"]
                aggt = st["aggt"]
                o1p = actpool.tile([128, 2, 512], BF16, tag="o1", bufs=2)
                ps = mm3ps.tile([128, 1024], F32, tag="mm3")
                for m in range(2):
                    nc.tensor.matmul(ps[:, m * 512:m * 512 + TT],
                                     wo1b[:, m, :], aggt[:, 0:TT],
                                     start=True, stop=True)
                if TT == 512:
                    nc.scalar.activation(o1p[:].rearrange("p m c -> p (m c)"),
                                         ps[:], AF.Prelu, alpha=alpha[:, 2:3])
                else:
                    for m in range(2):
                        nc.scalar.activation(o1p[:, m, 0:TT],
                                             ps[:, m * 512:m * 512 + TT],
                                             AF.Prelu, alpha=alpha[:, 2:3])
                o1_sb = [o1p[:, 0, :], o1p[:, 1, :]]
                o_tile = mm3ps.tile([128, 1024], F32, tag="mm3")
                o_ps = o_tile[0:NCLS, 0:TT]
                for k in range(2):
                    nc.tensor.matmul(o_ps, wo2b[:, k, :], o1_sb[k][0:128, 0:TT],
                                     start=(k == 0), stop=(k == 1),
                                     skip_group_check=True)
                out_sb = actpool.tile([64, 512], F32, tag="out", bufs=2)
                nc.scalar.activation(out_sb[0:64, 0:TT], o_ps, AF.Copy)
                nc.sync.dma_start(outT.ap()[:, n0:n0 + TT],
                                  out_sb[0:64, 0:TT])

            import contextlib
            loop_cm = tc.For_i(0, loop_k) if loop_k else contextlib.nullcontext()
            NT = len(TILES)
            states = {}
            with loop_cm:
              for _rep in range(unroll):
                for t in range(NT + 7):
                    if t < NT:
                        n0, TT = TILES[t]
                        states[t] = {"n0": n0, "TT": TT}
                        s_load(states[t])
                    if 1 <= t < NT + 1:
                        s_trans(states[t - 1])
                    if 5 <= t < NT + 5:
                        s_agg(states[t - 5])
                    if 2 <= t < NT + 2:
                        s_h1(states[t - 2])
                    if 3 <= t < NT + 3:
                        s_score(states[t - 3])
                    if 6 <= t < NT + 6:
                        s_aggt(states[t - 6])
                    if 7 <= t < NT + 7:
                        s_out(states[t - 7])
                    if 4 <= t < NT + 4:
                        s_soft(states[t - 4])
                    if 2 <= t < NT + 2:
                        s_jk(states[t - 2])
                    if 7 <= t < NT + 7:
                        del states[t - 7]

    nc.compile()
    return nc


def _get_program():
    if "nc" not in _CACHE:
        _CACHE["nc"] = _build_program()
    return _CACHE["nc"]


def kernel(**inputs):
    nc = _get_program()

    feats = np.asarray(inputs["feats"], dtype=np.float32)
    pad = NCORES * NPC - feats.shape[1]
    feats_p = np.pad(feats, ((0, 0), (0, pad), (0, 0)))

    def scal(name):
        return np.asarray(inputs[name], dtype=np.float32).reshape(1, 1)

    shared = {
        "W_jk1": np.ascontiguousarray(inputs["W_jk1"], dtype=np.float32),
        "W_jk2": np.ascontiguousarray(inputs["W_jk2"], dtype=np.float32),
        "w_att_ref": np.ascontiguousarray(inputs["w_att_ref"], dtype=np.float32),
        "w_att_x": np.ascontiguousarray(inputs["w_att_x"], dtype=np.float32),
        "W_o1": np.ascontiguousarray(inputs["W_o1"], dtype=np.float32),
        "W_o2": np.ascontiguousarray(inputs["W_o2"], dtype=np.float32),
        "a_jk": scal("a_jk"), "a_main": scal("a_main"), "a_out": scal("a_out"),
    }
    in_maps = []
    for c in range(NCORES):
        m = dict(shared)
        m["feats"] = np.ascontiguousarray(feats_p[:, c * NPC:(c + 1) * NPC, :])
        in_maps.append(m)

    res = run_bass_kernel_spmd(nc, in_maps, core_ids=list(range(NCORES)))
    out = np.concatenate(
        [np.asarray(res.results[c]["outT"]).T for c in range(NCORES)],
        axis=0)[:N]
    return np.ascontiguousarray(out, dtype=np.float32)


# revision 3
# speedup vs baseline: 4.9202x; 4.9202x over previous
"""JK-GAMLP forward on 8 Trainium2 NeuronCores (Bass/Tile) — fp8 DR, 8-stage.

Per core: node tiles of 512 (4 blocks of 128 on partitions).  The JK-MLP +
attention-score path runs in fp8e4m3 DoubleRow matmuls (K=256/instr, 0.5
cyc/row); weights pre-scaled by 32 (fp8 subnormal headroom), divided back
out in the activation scales.  The output path (hop aggregation -> output
FFN) is f32-accumulate with bf16 operands; output is stored transposed
([NCLS, N]) so store descriptors are 2KB, un-transposed on the host.

An 8-stage software pipeline: every engine's in-order queue gets work
whose cross-engine inputs completed in a PREVIOUS iteration, so nothing
stalls at queue heads.  Stage schedule per loop iteration t:
  LOAD(t) -> TRANS(t-1) -> AGG(t-5, Pool+DVE) -> H1(t-2) -> SCORE(t-3) ->
  AGGT(t-6) -> OUT(t-7) -> SOFT(t-4) -> JK(t-2)
"""
import numpy as np

import concourse.bacc as bacc
import concourse.mybir as mybir
import concourse.tile as tile
from concourse.bass_utils import run_bass_kernel_spmd

AF = mybir.ActivationFunctionType
ALU = mybir.AluOpType
AX = mybir.AxisListType
F32 = mybir.dt.float32
F32R = mybir.dt.float32r
BF16 = mybir.dt.bfloat16
FP8 = mybir.dt.float8e4
DR = mybir.MatmulPerfMode.DoubleRowSwInterleave

HOPS, F, HID, NCLS = 8, 128, 256, 64
N = 100000
NCORES = 8
NPC = 12544                       # nodes per core (padded: 8*12544 = 100352)
TILES = [(i * 512, 512) for i in range(24)] + [(12288, 256)]
WS = 32.0                         # fp8 weight pre-scale
ACT_XT_HOPS = (0, 2, 3, 4, 6)           # xt psum->sbuf copies on ACT; rest on DVE

_CACHE = {}


def _build_program(loop_k=None, unroll=1):
    nc = bacc.Bacc("TRN2", target_bir_lowering=False, debug=False,
                   num_devices=NCORES)

    feats = nc.dram_tensor("feats", [HOPS, NPC, F], F32R, kind="ExternalInput")
    W_jk1 = nc.dram_tensor("W_jk1", [HOPS * F, HID], F32, kind="ExternalInput")
    W_jk2 = nc.dram_tensor("W_jk2", [HID, HID], F32, kind="ExternalInput")
    w_att_ref = nc.dram_tensor("w_att_ref", [HID], F32, kind="ExternalInput")
    w_att_x = nc.dram_tensor("w_att_x", [F], F32, kind="ExternalInput")
    W_o1 = nc.dram_tensor("W_o1", [F, HID], F32, kind="ExternalInput")
    W_o2 = nc.dram_tensor("W_o2", [HID, NCLS], F32, kind="ExternalInput")
    a_jk = nc.dram_tensor("a_jk", [1, 1], F32, kind="ExternalInput")
    a_main = nc.dram_tensor("a_main", [1, 1], F32, kind="ExternalInput")
    a_out = nc.dram_tensor("a_out", [1, 1], F32, kind="ExternalInput")
    outT = nc.dram_tensor("outT", [NCLS, NPC], F32, kind="ExternalOutput")

    ident = nc.inline_tensor(np.eye(128, dtype=np.float32), name="ident")

    with tile.TileContext(nc) as tc:
        with tc.tile_pool(name="const", bufs=1) as cpool, \
             tc.tile_pool(name="x", bufs=34) as xpool, \
             tc.tile_pool(name="xt", bufs=13) as xtpool, \
             tc.tile_pool(name="act", bufs=3) as actpool, \
             tc.tile_pool(name="sm", bufs=4) as smpool, \
             tc.tile_pool(name="aggtmp", bufs=2) as aggtmpool, \
             tc.tile_pool(name="aggfin", bufs=12) as aggfpool, \
             tc.tile_pool(name="xt_ps", bufs=2, space="PSUM") as xtps, \
             tc.tile_pool(name="mm_ps", bufs=2, space="PSUM") as mmps, \
             tc.tile_pool(name="mm3_ps", bufs=1, space="PSUM") as mm3ps:

            # ---------------- setup: weights + constants ----------------
            id_sb = cpool.tile([128, 128], F32)
            nc.sync.dma_start(id_sb[:], ident[:])
            id_bf = cpool.tile([128, 128], BF16)
            nc.vector.tensor_copy(id_bf[:], id_sb[:])

            w1f = cpool.tile([128, HOPS, 2, 128], F32)
            nc.sync.dma_start(
                w1f[:], W_jk1.ap().rearrange("(h p) (m c) -> p h m c",
                                             p=128, m=2))
            # SwInterleave DR layout [p, j, m, 2c]: stored col 2v+i =
            # k-tile i (hop 2j+i), logical column 127-v, per the hardware
            # dual-fp8 LdWeights ordering.
            w1q = cpool.tile([128, 4, 2, 256], FP8)
            for j in range(4):
                for m in range(2):
                    for i in range(2):
                        nc.vector.tensor_scalar(
                            w1q[:, j, m, i::2],
                            w1f[:, 2 * j + i, m, ::-1],
                            WS, None, op0=ALU.mult)

            w2f = cpool.tile([128, 2, 2, 128], F32)
            nc.sync.dma_start(
                w2f[:], W_jk2.ap().rearrange("(k p) (m c) -> p k m c",
                                             p=128, m=2))
            # SwInterleave DR layout [p, m, 2c]
            w2q = cpool.tile([128, 2, 256], FP8)
            for m in range(2):
                for i in range(2):
                    nc.vector.tensor_scalar(
                        w2q[:, m, i::2], w2f[:, i, m, ::-1],
                        WS, None, op0=ALU.mult)

            wo1f = cpool.tile([128, 2, 128], F32)
            nc.sync.dma_start(
                wo1f[:], W_o1.ap().rearrange("p (m c) -> p m c", m=2))
            wo1b = cpool.tile([128, 2, 128], BF16)
            nc.vector.tensor_copy(wo1b[:], wo1f[:])

            wo2f = cpool.tile([128, 2, NCLS], F32)
            nc.sync.dma_start(
                wo2f[:], W_o2.ap().rearrange("(k p) c -> p k c", p=128))
            wo2b = cpool.tile([128, 2, NCLS], BF16)
            nc.vector.tensor_copy(wo2b[:], wo2f[:])

            wreff = cpool.tile([128, 2], F32)
            nc.sync.dma_start(wreff[:],
                              w_att_ref.ap().rearrange("(k p) -> p k", p=128))
            # SwInterleave, M padded to 128 (matches the known-good w1q
            # LdWeights shape): logical cols 0..7 = wref chunk i
            # (replicated), rest zero; stored col 2v+i = logical 127-v
            wrefq = cpool.tile([128, 256], FP8)
            nc.vector.memset(wrefq[:], 0.0)
            for i in range(2):
                for v in range(120, 128):
                    nc.vector.tensor_scalar(wrefq[:, 2 * v + i:2 * v + i + 1],
                                            wreff[:, i:i + 1], WS, None,
                                            op0=ALU.mult)

            watxf = cpool.tile([128, 1], F32)
            nc.sync.dma_start(watxf[:],
                              w_att_x.ap().rearrange("(p o) -> p o", o=1))
            # SwInterleave [p, j, 256], M=128: hop 2j+i diag at logical
            # col 2j+i -> stored col 2*(127-(2j+i))+i = 254-4j-i
            watxq = cpool.tile([128, 4, 256], FP8)
            nc.vector.memset(watxq[:], 0.0)
            for j in range(4):
                for i in range(2):
                    col = 254 - 4 * j - i
                    nc.vector.tensor_scalar(watxq[:, j, col:col + 1],
                                            watxf[:], WS, None, op0=ALU.mult)

            al_f = cpool.tile([1, 3], F32)
            nc.sync.dma_start(al_f[0:1, 0:1], a_jk[:])
            nc.sync.dma_start(al_f[0:1, 1:2], a_main[:])
            nc.sync.dma_start(al_f[0:1, 2:3], a_out[:])
            ones_sb = cpool.tile([1, 128], F32)
            nc.vector.memset(ones_sb[:], 1.0)
            half_col = cpool.tile([128, 1], F32)
            nc.vector.memset(half_col[:], 0.5)
            al_ps = mmps.tile([128, 3], F32, tag="mm")
            nc.tensor.matmul(al_ps[:], ones_sb[:], al_f[:],
                             start=True, stop=True)
            alpha = cpool.tile([128, 3], F32)
            nc.scalar.activation(alpha[:], al_ps[:], AF.Copy)

            # ---------------- pipeline stages ----------------

            def s_load(st):
                n0, TT = st["n0"], st["TT"]
                B = TT // 128
                x_blocks = []
                for b in range(B):
                    xb = xpool.tile([128, HOPS, 128], F32R, tag="x",
                                    name=f"xb{b}")
                    nc.sync.dma_start(
                        xb[:],
                        feats.ap()[:, n0 + b * 128:n0 + (b + 1) * 128, :]
                        .rearrange("h p f -> p h f"))
                    x_blocks.append(xb)
                st["x"] = x_blocks

            def s_trans(st):
                TT = st["TT"]
                B = TT // 128
                x = st["x"]
                xt_pairs = [xtpool.tile([128, 2, 512], FP8, tag="xt",
                                        name=f"xtp{j}")
                            for j in range(4)]
                for h in range(HOPS):
                    ps = xtps.tile([128, 512], F32R, tag="xtps")
                    for b in range(B):
                        nc.tensor.transpose(ps[:, b * 128:(b + 1) * 128],
                                            x[b][:, h, :],
                                            id_sb[:].bitcast(F32R))
                    dst = xt_pairs[h // 2][:, h % 2, 0:TT]
                    if h in ACT_XT_HOPS:
                        nc.scalar.activation(dst, ps[:, 0:TT], AF.Copy)
                    else:
                        nc.vector.tensor_copy(dst, ps[:, 0:TT])
                st["xt"] = xt_pairs

            def s_agg(st):
                TT = st["TT"]
                B = TT // 128
                x, ew = st["x"], st["ew"]
                agg_last = [None] * B
                for b in range(B):
                    cur = aggtmpool.tile([128, 128], F32, tag=f"aggd{b}",
                                         bufs=2, name=f"aggd{b}_0")
                    nc.vector.tensor_scalar(cur[:], x[b][:, 0, :].bitcast(F32),
                                            ew[:, b, 0:1], None, op0=ALU.mult)
                    for h in range(1, HOPS):
                        if h == HOPS - 1:
                            nxt = aggfpool.tile([128, 128], BF16, tag="aggf")
                        else:
                            nxt = aggtmpool.tile([128, 128], F32,
                                                 tag=f"aggd{b}", bufs=2,
                                                 name=f"aggd{b}_{h}")
                        nc.vector.scalar_tensor_tensor(
                            nxt[:], x[b][:, h, :].bitcast(F32),
                            ew[:, b, h:h + 1], cur[:],
                            op0=ALU.mult, op1=ALU.add)
                        cur = nxt
                    agg_last[b] = cur
                st["agg"] = agg_last

            def s_h1(st):
                TT = st["TT"]
                xt_pairs = st["xt"]
                h1_pair = actpool.tile([128, 2, 512], FP8, tag="h1", bufs=2)
                ps = mmps.tile([128, 1024], F32, tag="mm")
                for m in range(2):
                    for j in range(4):
                        nc.tensor.matmul(ps[:, m * 512:m * 512 + TT],
                                         w1q[:, j, m, :],
                                         xt_pairs[j][:, :, 0:TT],
                                         start=(j == 0), stop=(j == 3),
                                         perf_mode=DR)
                if TT == 512:
                    nc.scalar.activation(h1_pair[:].rearrange("p m c -> p (m c)"),
                                         ps[:], AF.Prelu, alpha=alpha[:, 0:1],
                                         scale=1.0 / WS)
                else:
                    for m in range(2):
                        nc.scalar.activation(h1_pair[:, m, 0:TT],
                                             ps[:, m * 512:m * 512 + TT],
                                             AF.Prelu, alpha=alpha[:, 0:1],
                                             scale=1.0 / WS)
                st["h1"] = h1_pair

            def s_jk(st):
                TT = st["TT"]
                h1_pair = st["h1"]
                jk_pair = actpool.tile([128, 2, 512], FP8, tag="jk", bufs=2)
                ps = mmps.tile([128, 1024], F32, tag="mm")
                for m in range(2):
                    nc.tensor.matmul(ps[:, m * 512:m * 512 + TT], w2q[:, m, :],
                                     h1_pair[:, :, 0:TT],
                                     start=True, stop=True, perf_mode=DR)
                if TT == 512:
                    nc.scalar.activation(jk_pair[:].rearrange("p m c -> p (m c)"),
                                         ps[:], AF.Prelu, alpha=alpha[:, 1:2],
                                         scale=1.0 / WS)
                else:
                    for m in range(2):
                        nc.scalar.activation(jk_pair[:, m, 0:TT],
                                             ps[:, m * 512:m * 512 + TT],
                                             AF.Prelu, alpha=alpha[:, 1:2],
                                             scale=1.0 / WS)
                st["jk"] = jk_pair

            def s_score(st):
                TT = st["TT"]
                xt_pairs, jk_pair = st["xt"], st["jk"]
                s_tile = mmps.tile([128, 1024], F32, tag="mm")
                s_ps = s_tile[0:128, 0:TT]
                nc.tensor.matmul(s_ps, wrefq[:], jk_pair[:, :, 0:TT],
                                 start=True, stop=False, perf_mode=DR,
                                 skip_group_check=True)
                for j in range(4):
                    nc.tensor.matmul(s_ps, watxq[:, j, :],
                                     xt_pairs[j][:, :, 0:TT],
                                     start=False, stop=(j == 3), perf_mode=DR,
                                     skip_group_check=True)
                sg = smpool.tile([8, 512], F32, tag="sg")
                nc.scalar.activation(sg[0:8, 0:TT], s_tile[0:8, 0:TT],
                                     AF.Tanh, scale=0.5 / WS)
                st["sg"] = sg

            def s_soft(st):
                TT = st["TT"]
                B = TT // 128
                sg = st["sg"]
                e_ps = mmps.tile([128, 1024], F32, tag="mm")
                for b in range(B):
                    nc.tensor.transpose(e_ps[:, b * 8:(b + 1) * 8],
                                        sg[0:8, b * 128:(b + 1) * 128],
                                        id_sb[0:8, 0:8])
                e_sb = smpool.tile([128, 4, 8], F32, tag="e")
                nc.scalar.activation(e_sb[:, 0:B, :], e_ps[:, 0:B * 8],
                                     AF.Exp, scale=0.5, bias=half_col[:])
                esum = smpool.tile([128, 4], F32, tag="esum")
                nc.vector.tensor_reduce(esum[:, 0:B], e_sb[:, 0:B, :],
                                        axis=AX.X, op=ALU.add)
                r_sb = smpool.tile([128, 4], F32, tag="r")
                nc.vector.reciprocal(r_sb[:, 0:B], esum[:, 0:B])
                ew = smpool.tile([128, 4, 8], F32, tag="ew")
                for b in range(B):
                    nc.vector.tensor_scalar(ew[:, b, :], e_sb[:, b, :],
                                            r_sb[:, b:b + 1], None,
                                            op0=ALU.mult)
                st["ew"] = ew

            def s_aggt(st):
                TT = st["TT"]
                B = TT // 128
                a_ps = mm3ps.tile([128, 1024], BF16, tag="mm3")
                for b in range(B):
                    nc.tensor.transpose(a_ps[:, b * 128:(b + 1) * 128],
                                        st["agg"][b][:], id_bf[:])
                aggt = actpool.tile([128, 512], BF16, tag="aggt", bufs=3)
                nc.vector.tensor_copy(aggt[:, 0:TT], a_ps[:, 0:TT])
                st["aggt"] = aggt

            def s_out(st):
                n0, TT = st["n0"], st["TT"]
                aggt = st["aggt"]
                o1p = actpool.tile([128, 2, 512], BF16, tag="o1", bufs=2)
                ps = mm3ps.tile([128, 1024], F32, tag="mm3")
                for m in range(2):
                    nc.tensor.matmul(ps[:, m * 512:m * 512 + TT],
                                     wo1b[:, m, :], aggt[:, 0:TT],
                                     start=True, stop=True)
                if TT == 512:
                    nc.scalar.activation(o1p[:].rearrange("p m c -> p (m c)"),
                                         ps[:], AF.Prelu, alpha=alpha[:, 2:3])
                else:
                    for m in range(2):
                        nc.scalar.activation(o1p[:, m, 0:TT],
                                             ps[:, m * 512:m * 512 + TT],
                                             AF.Prelu, alpha=alpha[:, 2:3])
                o1_sb = [o1p[:, 0, :], o1p[:, 1, :]]
                o_tile = mm3ps.tile([128, 1024], F32, tag="mm3")
                o_ps = o_tile[0:NCLS, 0:TT]
                for k in range(2):
                    nc.tensor.matmul(o_ps, wo2b[:, k, :], o1_sb[k][0:128, 0:TT],
                                     start=(k == 0), stop=(k == 1),
                                     skip_group_check=True)
                out_sb = actpool.tile([64, 512], F32, tag="out", bufs=2)
                nc.scalar.activation(out_sb[0:64, 0:TT], o_ps, AF.Copy)
                nc.sync.dma_start(outT.ap()[:, n0:n0 + TT],
                                  out_sb[0:64, 0:TT])

            import contextlib
            loop_cm = tc.For_i(0, loop_k) if loop_k else contextlib.nullcontext()
            NT = len(TILES)
            states = {}
            with loop_cm:
              for _rep in range(unroll):
                for t in range(NT + 7):
                    if t < NT:
                        n0, TT = TILES[t]
                        states[t] = {"n0": n0, "TT": TT}
                        s_load(states[t])
                    if 1 <= t < NT + 1:
                        s_trans(states[t - 1])
                    if 5 <= t < NT + 5:
                        s_agg(states[t - 5])
                    if 2 <= t < NT + 2:
                        s_h1(states[t - 2])
                    if 3 <= t < NT + 3:
                        s_score(states[t - 3])
                    if 6 <= t < NT + 6:
                        s_aggt(states[t - 6])
                    if 7 <= t < NT + 7:
                        s_out(states[t - 7])
                    if 4 <= t < NT + 4:
                        s_soft(states[t - 4])
                    if 2 <= t < NT + 2:
                        s_jk(states[t - 2])
                    if 7 <= t < NT + 7:
                        del states[t - 7]

    nc.compile()
    return nc


def _get_program():
    if "nc" not in _CACHE:
        _CACHE["nc"] = _build_program()
    return _CACHE["nc"]


def kernel(**inputs):
    nc = _get_program()

    feats = np.asarray(inputs["feats"], dtype=np.float32)
    pad = NCORES * NPC - feats.shape[1]
    feats_p = np.pad(feats, ((0, 0), (0, pad), (0, 0)))

    def scal(name):
        return np.asarray(inputs[name], dtype=np.float32).reshape(1, 1)

    shared = {
        "W_jk1": np.ascontiguousarray(inputs["W_jk1"], dtype=np.float32),
        "W_jk2": np.ascontiguousarray(inputs["W_jk2"], dtype=np.float32),
        "w_att_ref": np.ascontiguousarray(inputs["w_att_ref"], dtype=np.float32),
        "w_att_x": np.ascontiguousarray(inputs["w_att_x"], dtype=np.float32),
        "W_o1": np.ascontiguousarray(inputs["W_o1"], dtype=np.float32),
        "W_o2": np.ascontiguousarray(inputs["W_o2"], dtype=np.float32),
        "a_jk": scal("a_jk"), "a_main": scal("a_main"), "a_out": scal("a_out"),
    }
    in_maps = []
    for c in range(NCORES):
        m = dict(shared)
        m["feats"] = np.ascontiguousarray(feats_p[:, c * NPC:(c + 1) * NPC, :])
        in_maps.append(m)

    res = run_bass_kernel_spmd(nc, in_maps, core_ids=list(range(NCORES)))
    out = np.concatenate(
        [np.asarray(res.results[c]["outT"]).T for c in range(NCORES)],
        axis=0)[:N]
    return np.ascontiguousarray(out, dtype=np.float32)


# revision 4
# speedup vs baseline: 9.6477x; 1.9609x over previous
"""JK-GAMLP forward on 8 Trainium2 NeuronCores (Bass/Tile) — fp8 DR, 8-stage.

Per core: node tiles of 512 (4 blocks of 128 on partitions).  The JK-MLP +
attention-score path runs in fp8e4m3 DoubleRow matmuls (K=256/instr, 0.5
cyc/row); weights pre-scaled by 32 (fp8 subnormal headroom), divided back
out in the activation scales.  The output path (hop aggregation -> output
FFN) is f32-accumulate with bf16 operands; output is stored transposed
([NCLS, N]) so store descriptors are 2KB, un-transposed on the host.

An 8-stage software pipeline: every engine's in-order queue gets work
whose cross-engine inputs completed in a PREVIOUS iteration, so nothing
stalls at queue heads.  Stage schedule per loop iteration t:
  LOAD(t) -> TRANS(t-1) -> AGG(t-5, Pool+DVE) -> H1(t-2) -> SCORE(t-3) ->
  AGGT(t-6) -> OUT(t-7) -> SOFT(t-4) -> JK(t-2)
"""
import numpy as np

import concourse.bacc as bacc
import concourse.mybir as mybir
import concourse.tile as tile
from concourse.bass_utils import run_bass_kernel_spmd

AF = mybir.ActivationFunctionType
ALU = mybir.AluOpType
AX = mybir.AxisListType
F32 = mybir.dt.float32
F32R = mybir.dt.float32r
BF16 = mybir.dt.bfloat16
FP8 = mybir.dt.float8e4
DR = mybir.MatmulPerfMode.DoubleRowSwInterleave

HOPS, F, HID, NCLS = 8, 128, 256, 64
N = 100000
NCORES = 8
NPC = 12544                       # nodes per core (padded: 8*12544 = 100352)
TILES = [(i * 512, 512) for i in range(24)] + [(12288, 256)]
WS = 32.0                         # fp8 weight pre-scale
ACT_XT_HOPS = (0, 2, 3, 4, 6)           # xt psum->sbuf copies on ACT; rest on DVE

_CACHE = {}


def _build_program(loop_k=None, unroll=1):
    nc = bacc.Bacc("TRN2", target_bir_lowering=False, debug=False,
                   num_devices=NCORES)

    feats = nc.dram_tensor("feats", [NPC, HOPS, F], BF16, kind="ExternalInput")
    W_jk1 = nc.dram_tensor("W_jk1", [HOPS * F, HID], F32, kind="ExternalInput")
    W_jk2 = nc.dram_tensor("W_jk2", [HID, HID], F32, kind="ExternalInput")
    w_att_ref = nc.dram_tensor("w_att_ref", [HID], F32, kind="ExternalInput")
    w_att_x = nc.dram_tensor("w_att_x", [F], F32, kind="ExternalInput")
    W_o1 = nc.dram_tensor("W_o1", [F, HID], F32, kind="ExternalInput")
    W_o2 = nc.dram_tensor("W_o2", [HID, NCLS], F32, kind="ExternalInput")
    a_jk = nc.dram_tensor("a_jk", [1, 1], F32, kind="ExternalInput")
    a_main = nc.dram_tensor("a_main", [1, 1], F32, kind="ExternalInput")
    a_out = nc.dram_tensor("a_out", [1, 1], F32, kind="ExternalInput")
    outT = nc.dram_tensor("outT", [NCLS, NPC], F32, kind="ExternalOutput")

    ident = nc.inline_tensor(np.eye(128, dtype=np.float32), name="ident")

    with tile.TileContext(nc) as tc:
        with tc.tile_pool(name="const", bufs=1) as cpool, \
             tc.tile_pool(name="x", bufs=34) as xpool, \
             tc.tile_pool(name="xt", bufs=13) as xtpool, \
             tc.tile_pool(name="act", bufs=3) as actpool, \
             tc.tile_pool(name="sm", bufs=4) as smpool, \
             tc.tile_pool(name="aggtmp", bufs=2) as aggtmpool, \
             tc.tile_pool(name="aggfin", bufs=12) as aggfpool, \
             tc.tile_pool(name="xt_ps", bufs=2, space="PSUM") as xtps, \
             tc.tile_pool(name="mm_ps", bufs=2, space="PSUM") as mmps, \
             tc.tile_pool(name="mm3_ps", bufs=1, space="PSUM") as mm3ps:

            # ---------------- setup: weights + constants ----------------
            id_sb = cpool.tile([128, 128], F32)
            nc.sync.dma_start(id_sb[:], ident[:])
            id_bf = cpool.tile([128, 128], BF16)
            nc.vector.tensor_copy(id_bf[:], id_sb[:])

            w1f = cpool.tile([128, HOPS, 2, 128], F32)
            nc.sync.dma_start(
                w1f[:], W_jk1.ap().rearrange("(h p) (m c) -> p h m c",
                                             p=128, m=2))
            # SwInterleave DR layout [p, j, m, 2c]: stored col 2v+i =
            # k-tile i (hop 2j+i), logical column 127-v, per the hardware
            # dual-fp8 LdWeights ordering.
            w1q = cpool.tile([128, 4, 2, 256], FP8)
            for j in range(4):
                for m in range(2):
                    for i in range(2):
                        nc.vector.tensor_scalar(
                            w1q[:, j, m, i::2],
                            w1f[:, 2 * j + i, m, ::-1],
                            WS, None, op0=ALU.mult)

            w2f = cpool.tile([128, 2, 2, 128], F32)
            nc.sync.dma_start(
                w2f[:], W_jk2.ap().rearrange("(k p) (m c) -> p k m c",
                                             p=128, m=2))
            # SwInterleave DR layout [p, m, 2c]
            w2q = cpool.tile([128, 2, 256], FP8)
            for m in range(2):
                for i in range(2):
                    nc.vector.tensor_scalar(
                        w2q[:, m, i::2], w2f[:, i, m, ::-1],
                        WS, None, op0=ALU.mult)

            wo1f = cpool.tile([128, 2, 128], F32)
            nc.sync.dma_start(
                wo1f[:], W_o1.ap().rearrange("p (m c) -> p m c", m=2))
            wo1b = cpool.tile([128, 2, 128], BF16)
            nc.vector.tensor_copy(wo1b[:], wo1f[:])

            wo2f = cpool.tile([128, 2, NCLS], F32)
            nc.sync.dma_start(
                wo2f[:], W_o2.ap().rearrange("(k p) c -> p k c", p=128))
            wo2b = cpool.tile([128, 2, NCLS], BF16)
            nc.vector.tensor_copy(wo2b[:], wo2f[:])

            wreff = cpool.tile([128, 2], F32)
            nc.sync.dma_start(wreff[:],
                              w_att_ref.ap().rearrange("(k p) -> p k", p=128))
            # SwInterleave, M padded to 128 (matches the known-good w1q
            # LdWeights shape): logical cols 0..7 = wref chunk i
            # (replicated), rest zero; stored col 2v+i = logical 127-v
            wrefq = cpool.tile([128, 256], FP8)
            nc.vector.memset(wrefq[:], 0.0)
            for i in range(2):
                for v in range(120, 128):
                    nc.vector.tensor_scalar(wrefq[:, 2 * v + i:2 * v + i + 1],
                                            wreff[:, i:i + 1], WS, None,
                                            op0=ALU.mult)

            watxf = cpool.tile([128, 1], F32)
            nc.sync.dma_start(watxf[:],
                              w_att_x.ap().rearrange("(p o) -> p o", o=1))
            # SwInterleave [p, j, 256], M=128: hop 2j+i diag at logical
            # col 2j+i -> stored col 2*(127-(2j+i))+i = 254-4j-i
            watxq = cpool.tile([128, 4, 256], FP8)
            nc.vector.memset(watxq[:], 0.0)
            for j in range(4):
                for i in range(2):
                    col = 254 - 4 * j - i
                    nc.vector.tensor_scalar(watxq[:, j, col:col + 1],
                                            watxf[:], WS, None, op0=ALU.mult)

            al_f = cpool.tile([1, 3], F32)
            nc.sync.dma_start(al_f[0:1, 0:1], a_jk[:])
            nc.sync.dma_start(al_f[0:1, 1:2], a_main[:])
            nc.sync.dma_start(al_f[0:1, 2:3], a_out[:])
            ones_sb = cpool.tile([1, 128], F32)
            nc.vector.memset(ones_sb[:], 1.0)
            half_col = cpool.tile([128, 1], F32)
            nc.vector.memset(half_col[:], 0.5)
            al_ps = mmps.tile([128, 3], F32, tag="mm")
            nc.tensor.matmul(al_ps[:], ones_sb[:], al_f[:],
                             start=True, stop=True)
            alpha = cpool.tile([128, 3], F32)
            nc.scalar.activation(alpha[:], al_ps[:], AF.Copy)

            # ---------------- pipeline stages ----------------

            def s_load(st):
                n0, TT = st["n0"], st["TT"]
                B = TT // 128
                x_blocks = []
                for b in range(B):
                    xb = xpool.tile([128, HOPS, 128], BF16, tag="x",
                                    name=f"xb{b}")
                    nc.sync.dma_start(
                        xb[:],
                        feats.ap()[n0 + b * 128:n0 + (b + 1) * 128, :, :])
                    x_blocks.append(xb)
                st["x"] = x_blocks

            def s_trans(st):
                TT = st["TT"]
                B = TT // 128
                x = st["x"]
                xt_pairs = [xtpool.tile([128, 2, 512], FP8, tag="xt",
                                        name=f"xtp{j}")
                            for j in range(4)]
                for h in range(HOPS):
                    ps = xtps.tile([128, 512], BF16, tag="xtps")
                    for b in range(B):
                        nc.tensor.transpose(ps[:, b * 128:(b + 1) * 128],
                                            x[b][:, h, :], id_bf[:])
                    dst = xt_pairs[h // 2][:, h % 2, 0:TT]
                    if h in ACT_XT_HOPS:
                        nc.scalar.activation(dst, ps[:, 0:TT], AF.Copy)
                    else:
                        nc.vector.tensor_copy(dst, ps[:, 0:TT])
                st["xt"] = xt_pairs

            def s_agg(st):
                TT = st["TT"]
                B = TT // 128
                x, ew = st["x"], st["ew"]
                agg_last = [None] * B
                for b in range(B):
                    cur = aggtmpool.tile([128, 128], F32, tag=f"aggd{b}",
                                         bufs=2, name=f"aggd{b}_0")
                    nc.vector.tensor_scalar(cur[:], x[b][:, 0, :],
                                            ew[:, b, 0:1], None, op0=ALU.mult)
                    for h in range(1, HOPS):
                        if h == HOPS - 1:
                            nxt = aggfpool.tile([128, 128], BF16, tag="aggf")
                        else:
                            nxt = aggtmpool.tile([128, 128], F32,
                                                 tag=f"aggd{b}", bufs=2,
                                                 name=f"aggd{b}_{h}")
                        nc.vector.scalar_tensor_tensor(
                            nxt[:], x[b][:, h, :],
                            ew[:, b, h:h + 1], cur[:],
                            op0=ALU.mult, op1=ALU.add)
                        cur = nxt
                    agg_last[b] = cur
                st["agg"] = agg_last

            def s_h1(st):
                TT = st["TT"]
                xt_pairs = st["xt"]
                h1_pair = actpool.tile([128, 2, 512], FP8, tag="h1", bufs=2)
                ps = mmps.tile([128, 1024], F32, tag="mm")
                for m in range(2):
                    for j in range(4):
                        nc.tensor.matmul(ps[:, m * 512:m * 512 + TT],
                                         w1q[:, j, m, :],
                                         xt_pairs[j][:, :, 0:TT],
                                         start=(j == 0), stop=(j == 3),
                                         perf_mode=DR)
                if TT == 512:
                    nc.scalar.activation(h1_pair[:].rearrange("p m c -> p (m c)"),
                                         ps[:], AF.Prelu, alpha=alpha[:, 0:1],
                                         scale=1.0 / WS)
                else:
                    for m in range(2):
                        nc.scalar.activation(h1_pair[:, m, 0:TT],
                                             ps[:, m * 512:m * 512 + TT],
                                             AF.Prelu, alpha=alpha[:, 0:1],
                                             scale=1.0 / WS)
                st["h1"] = h1_pair

            def s_jk(st):
                TT = st["TT"]
                h1_pair = st["h1"]
                jk_pair = actpool.tile([128, 2, 512], FP8, tag="jk", bufs=2)
                ps = mmps.tile([128, 1024], F32, tag="mm")
                for m in range(2):
                    nc.tensor.matmul(ps[:, m * 512:m * 512 + TT], w2q[:, m, :],
                                     h1_pair[:, :, 0:TT],
                                     start=True, stop=True, perf_mode=DR)
                if TT == 512:
                    nc.scalar.activation(jk_pair[:].rearrange("p m c -> p (m c)"),
                                         ps[:], AF.Prelu, alpha=alpha[:, 1:2],
                                         scale=1.0 / WS)
                else:
                    for m in range(2):
                        nc.scalar.activation(jk_pair[:, m, 0:TT],
                                             ps[:, m * 512:m * 512 + TT],
                                             AF.Prelu, alpha=alpha[:, 1:2],
                                             scale=1.0 / WS)
                st["jk"] = jk_pair

            def s_score(st):
                TT = st["TT"]
                xt_pairs, jk_pair = st["xt"], st["jk"]
                s_tile = mmps.tile([128, 1024], F32, tag="mm")
                s_ps = s_tile[0:128, 0:TT]
                nc.tensor.matmul(s_ps, wrefq[:], jk_pair[:, :, 0:TT],
                                 start=True, stop=False, perf_mode=DR,
                                 skip_group_check=True)
                for j in range(4):
                    nc.tensor.matmul(s_ps, watxq[:, j, :],
                                     xt_pairs[j][:, :, 0:TT],
                                     start=False, stop=(j == 3), perf_mode=DR,
                                     skip_group_check=True)
                sg = smpool.tile([8, 512], F32, tag="sg")
                nc.scalar.activation(sg[0:8, 0:TT], s_tile[0:8, 0:TT],
                                     AF.Tanh, scale=0.5 / WS)
                st["sg"] = sg

            def s_soft(st):
                TT = st["TT"]
                B = TT // 128
                sg = st["sg"]
                e_ps = mmps.tile([128, 1024], F32, tag="mm")
                for b in range(B):
                    nc.tensor.transpose(e_ps[:, b * 8:(b + 1) * 8],
                                        sg[0:8, b * 128:(b + 1) * 128],
                                        id_sb[0:8, 0:8])
                e_sb = smpool.tile([128, 4, 8], F32, tag="e")
                nc.scalar.activation(e_sb[:, 0:B, :], e_ps[:, 0:B * 8],
                                     AF.Exp, scale=0.5, bias=half_col[:])
                esum = smpool.tile([128, 4], F32, tag="esum")
                nc.vector.tensor_reduce(esum[:, 0:B], e_sb[:, 0:B, :],
                                        axis=AX.X, op=ALU.add)
                r_sb = smpool.tile([128, 4], F32, tag="r")
                nc.vector.reciprocal(r_sb[:, 0:B], esum[:, 0:B])
                ew = smpool.tile([128, 4, 8], F32, tag="ew")
                for b in range(B):
                    nc.vector.tensor_scalar(ew[:, b, :], e_sb[:, b, :],
                                            r_sb[:, b:b + 1], None,
                                            op0=ALU.mult)
                st["ew"] = ew

            def s_aggt(st):
                TT = st["TT"]
                B = TT // 128
                a_ps = mm3ps.tile([128, 1024], BF16, tag="mm3")
                for b in range(B):
                    nc.tensor.transpose(a_ps[:, b * 128:(b + 1) * 128],
                                        st["agg"][b][:], id_bf[:])
                aggt = actpool.tile([128, 512], BF16, tag="aggt", bufs=3)
                nc.vector.tensor_copy(aggt[:, 0:TT], a_ps[:, 0:TT])
                st["aggt"] = aggt

            def s_out(st):
                n0, TT = st["n0"], st["TT"]
                aggt = st["aggt"]
                o1p = actpool.tile([128, 2, 512], BF16, tag="o1", bufs=2)
                ps = mm3ps.tile([128, 1024], F32, tag="mm3")
                for m in range(2):
                    nc.tensor.matmul(ps[:, m * 512:m * 512 + TT],
                                     wo1b[:, m, :], aggt[:, 0:TT],
                                     start=True, stop=True)
                if TT == 512:
                    nc.scalar.activation(o1p[:].rearrange("p m c -> p (m c)"),
                                         ps[:], AF.Prelu, alpha=alpha[:, 2:3])
                else:
                    for m in range(2):
                        nc.scalar.activation(o1p[:, m, 0:TT],
                                             ps[:, m * 512:m * 512 + TT],
                                             AF.Prelu, alpha=alpha[:, 2:3])
                o1_sb = [o1p[:, 0, :], o1p[:, 1, :]]
                o_tile = mm3ps.tile([128, 1024], F32, tag="mm3")
                o_ps = o_tile[0:NCLS, 0:TT]
                for k in range(2):
                    nc.tensor.matmul(o_ps, wo2b[:, k, :], o1_sb[k][0:128, 0:TT],
                                     start=(k == 0), stop=(k == 1),
                                     skip_group_check=True)
                out_sb = actpool.tile([64, 512], F32, tag="out", bufs=2)
                nc.scalar.activation(out_sb[0:64, 0:TT], o_ps, AF.Copy)
                nc.sync.dma_start(outT.ap()[:, n0:n0 + TT],
                                  out_sb[0:64, 0:TT])

            import contextlib
            loop_cm = tc.For_i(0, loop_k) if loop_k else contextlib.nullcontext()
            NT = len(TILES)
            states = {}
            with loop_cm:
              for _rep in range(unroll):
                for t in range(NT + 7):
                    if t < NT:
                        n0, TT = TILES[t]
                        states[t] = {"n0": n0, "TT": TT}
                        s_load(states[t])
                    if 1 <= t < NT + 1:
                        s_trans(states[t - 1])
                    if 5 <= t < NT + 5:
                        s_agg(states[t - 5])
                    if 2 <= t < NT + 2:
                        s_h1(states[t - 2])
                    if 3 <= t < NT + 3:
                        s_score(states[t - 3])
                    if 6 <= t < NT + 6:
                        s_aggt(states[t - 6])
                    if 7 <= t < NT + 7:
                        s_out(states[t - 7])
                    if 4 <= t < NT + 4:
                        s_soft(states[t - 4])
                    if 2 <= t < NT + 2:
                        s_jk(states[t - 2])
                    if 7 <= t < NT + 7:
                        del states[t - 7]

    nc.compile()
    return nc


def _get_program():
    if "nc" not in _CACHE:
        _CACHE["nc"] = _build_program()
    return _CACHE["nc"]


def kernel(**inputs):
    nc = _get_program()

    import ml_dtypes
    feats = np.asarray(inputs["feats"], dtype=np.float32)
    pad = NCORES * NPC - feats.shape[1]
    feats_p = np.pad(feats, ((0, 0), (0, pad), (0, 0)))
    # node-major bf16 staging: [N, HOPS, F], 2KB contiguous per node
    feats_p = np.ascontiguousarray(
        feats_p.transpose(1, 0, 2)).astype(ml_dtypes.bfloat16)

    def scal(name):
        return np.asarray(inputs[name], dtype=np.float32).reshape(1, 1)

    shared = {
        "W_jk1": np.ascontiguousarray(inputs["W_jk1"], dtype=np.float32),
        "W_jk2": np.ascontiguousarray(inputs["W_jk2"], dtype=np.float32),
        "w_att_ref": np.ascontiguousarray(inputs["w_att_ref"], dtype=np.float32),
        "w_att_x": np.ascontiguousarray(inputs["w_att_x"], dtype=np.float32),
        "W_o1": np.ascontiguousarray(inputs["W_o1"], dtype=np.float32),
        "W_o2": np.ascontiguousarray(inputs["W_o2"], dtype=np.float32),
        "a_jk": scal("a_jk"), "a_main": scal("a_main"), "a_out": scal("a_out"),
    }
    in_maps = []
    for c in range(NCORES):
        m = dict(shared)
        m["feats"] = np.ascontiguousarray(feats_p[c * NPC:(c + 1) * NPC])
        in_maps.append(m)

    res = run_bass_kernel_spmd(nc, in_maps, core_ids=list(range(NCORES)))
    out = np.concatenate(
        [np.asarray(res.results[c]["outT"]).T for c in range(NCORES)],
        axis=0)[:N]
    return np.ascontiguousarray(out, dtype=np.float32)
